# revision 2
# baseline (speedup 1.0000x reference)
"""DeepSeekMoE layer on 8 Trainium2 NeuronCores — host-routed version.

Problem (hardcoded): B=2, T=1024, C=1024, H=4096, E_routed=8 (top-2 sigmoid
gating), E_shared=2, fp32.

v2 design: the host computes rmsnorm + sigmoid top-2 routing (0.2% of the
FLOPs) and dispatches only the SELECTED (token, expert) work to the device:

  Core c runs two expert segments, SPMD-identical program shape:
    seg 0: shared expert c//4 on token quarter c%4          (512 tokens)
    seg 1: routed expert c on its selected tokens           (CAP slots)
  CAP = max routed-expert token count rounded up to 32; unused slots carry
  gate 0.  Total device work drops from 8*5*512 = 20480 token-passes to
  8*(512+CAP) ~ 8700 — a ~2.3x compute reduction vs the dense kernel.

Device kernel (per core): T-layout [C-partition, token-free].  Host sends
pre-normalized activations xn in bf16 (g folded into W1 on host, so shared
and routed use the same xn).  Per segment: W1 stationary tiles x xn moving
-> PSUM -> gelu(+b1) on ACT -> h (bf16) -> W2 stationary x h moving -> PSUM
-> eviction on DVE: out = (psum + b2) * gate  (gate row pre-broadcast by the
host; 1.0 for shared slots, normalized sigmoid gate for routed, 0 for pads).
Host scatters the routed partials back by token index and adds u.
"""
import contextlib
import os
import numpy as np

import concourse.bass as bass
import concourse.tile as tile
from concourse import bacc, mybir
from concourse import bass_utils
from concourse.alu_op_type import AluOpType

F32 = mybir.dt.float32
F32R = mybir.dt.float32r
BF16 = mybir.dt.bfloat16
AF = mybir.ActivationFunctionType
AX = mybir.AxisListType

B, T, C, H = 2, 1024, 1024, 4096
E_R, E_S = 8, 2
NCORES = 8
TOKC = 512          # shared-segment tokens per core
CK = C // 128       # 8 c-tiles
HK = H // 128       # 32 h-tiles
NMC = 16            # W1 m-chunks (each 2 h-tiles = 256 h cols)
NSEG = 2            # expert segments per core: shared, routed
EPS = 1.1920929e-07

_CACHE = {}


def _chunks(cap):
    """Token chunks (col offset within the routed segment, size<=512)."""
    out, t0 = [], 0
    while t0 < cap:
        out.append((t0, min(512, cap - t0)))
        t0 += 512
    return out


def _build_program(cap, loop_m=1):
    tt = TOKC + cap
    nc = bacc.Bacc("TRN2", target_bir_lowering=False, debug=False,
                   enable_asserts=False)

    d = {}
    d["xn"] = nc.dram_tensor("xn", [128, CK * tt], BF16, kind="ExternalInput").ap()
    d["w1"] = nc.dram_tensor("w1", [NSEG, NMC, 128, CK * 256], BF16, kind="ExternalInput").ap()
    d["w2"] = nc.dram_tensor("w2", [NSEG, HK // 2, 128, 2 * 1024], BF16, kind="ExternalInput").ap()
    d["b1"] = nc.dram_tensor("b1t", [128, NSEG * HK], F32, kind="ExternalInput").ap()
    d["b2c"] = nc.dram_tensor("b2c", [128, NSEG * CK], F32, kind="ExternalInput").ap()
    d["gw"] = nc.dram_tensor("gw", [128, tt], F32, kind="ExternalInput").ap()
    d["out"] = nc.dram_tensor("outT", [128, CK * tt], F32, kind="ExternalOutput").ap()

    with tile.TileContext(nc) as tc:
        with (
            tc.tile_pool(name="io", bufs=1) as io,
            tc.tile_pool(name="w1p", bufs=2) as w1p,
            tc.tile_pool(name="w2p", bufs=3) as w2p,
            tc.tile_pool(name="hp", bufs=64) as hp,
            tc.tile_pool(name="pp", bufs=8, space="PSUM") as pp,
        ):
            pools = dict(io=io, w1p=w1p, w2p=w2p, hp=hp, pp=pp)
            loop = tc.For_i(0, loop_m, 1) if loop_m > 1 else contextlib.nullcontext()
            with loop:
                _moe_body(nc, d, pools, cap)

    nc.compile()
    return nc


def _moe_body(nc, d, p, cap):
    io, w1p, w2p, hp, pp = p["io"], p["w1p"], p["w2p"], p["hp"], p["pp"]
    tt = TOKC + cap
    # segment -> list of (col offset within tt, chunk size)
    seg_chunks = [[(0, TOKC)], [(TOKC + t0, tc) for (t0, tc) in _chunks(cap)]]

    # ---- input loads ----
    xn = io.tile([128, CK * tt], BF16, tag="xn", name="xn")
    for k in range(CK):
        nc.sync.dma_start(xn[:, tt * k:tt * (k + 1)], d["xn"][:, tt * k:tt * (k + 1)])
    gw = io.tile([128, tt], F32, tag="gw", name="gw")
    nc.sync.dma_start(gw[:], d["gw"])
    b1 = io.tile([128, NSEG * HK], F32, tag="b1", name="b1")
    nc.sync.dma_start(b1[:], d["b1"])
    b2c = io.tile([128, NSEG * CK], F32, tag="b2c", name="b2c")
    nc.sync.dma_start(b2c[:], d["b2c"])

    out = io.tile([128, CK * tt], F32, tag="out", name="out")

    # ---- expert segments ----
    for e in range(NSEG):
        chunks = seg_chunks[e]
        # W1 phase: h[hh][chunk] = gelu(W1[:, hh]^T xn[chunk] + b1[hh])
        h_act = {}
        for mc in range(NMC):
            w1c = w1p.tile([128, CK * 256], BF16, tag="w1c", name=f"w1c_{e}_{mc}")
            half = CK * 256 // 2
            nc.sync.dma_start(w1c[:, :half], d["w1"][e, mc][:, :half])
            nc.sync.dma_start(w1c[:, half:], d["w1"][e, mc][:, half:])
            ph = {(m, ci): pp.tile([128, tc], F32, tag="pp",
                                   name=f"ph_{e}_{mc}_{m}_{ci}")
                  for m in range(2) for ci, (t0, tc) in enumerate(chunks)}
            for k in range(CK):
                for m in range(2):
                    for ci, (t0, tc) in enumerate(chunks):
                        nc.tensor.matmul(
                            ph[m, ci][:],
                            w1c[:, 256 * k + 128 * m:256 * k + 128 * (m + 1)],
                            xn[:, tt * k + t0:tt * k + t0 + tc],
                            start=(k == 0), stop=(k == CK - 1))
            for m in range(2):
                hh = 2 * mc + m
                for ci, (t0, tc) in enumerate(chunks):
                    ht = hp.tile([128, tc], BF16, tag=f"h{ci}",
                                 name=f"h_{e}_{hh}_{ci}")
                    nc.scalar.activation(ht[:], ph[m, ci][:], AF.Gelu,
                                         bias=b1[:, e * HK + hh:e * HK + hh + 1])
                    h_act[hh, ci] = ht
        # W2 phase, one token chunk at a time (8 PSUM banks per chunk)
        for ci, (t0, tc) in enumerate(chunks):
            py = [pp.tile([128, tc], F32, tag="pp", name=f"py_{e}_{ci}_{m}")
                  for m in range(CK)]
            for kk in range(HK // 2):
                w2s = w2p.tile([128, 2 * 1024], BF16, tag="w2s",
                               name=f"w2s_{e}_{ci}_{kk}")
                nc.sync.dma_start(w2s[:, :1024], d["w2"][e, kk][:, :1024])
                nc.sync.dma_start(w2s[:, 1024:], d["w2"][e, kk][:, 1024:])
                for k2 in range(2):
                    for m in range(CK):
                        nc.tensor.matmul(
                            py[m][:],
                            w2s[:, 1024 * k2 + 128 * m:1024 * k2 + 128 * (m + 1)],
                            h_act[2 * kk + k2, ci][:],
                            start=(kk == 0 and k2 == 0),
                            stop=(kk == HK // 2 - 1 and k2 == 1))
            # eviction: out = (psum + b2[c]) * gate
            for m in range(CK):
                nc.vector.scalar_tensor_tensor(
                    out[:, tt * m + t0:tt * m + t0 + tc], py[m][:],
                    b2c[:, e * CK + m:e * CK + m + 1],
                    gw[:, t0:t0 + tc],
                    AluOpType.add, AluOpType.mult)

    # ---- store (single DMA) ----
    nc.sync.dma_start(d["out"], out[:])


def _pack_expert(W1, b1, W2, b2, g):
    """Pack one expert's weights for the device layout (bf16)."""
    f = np.float32
    bf = mybir.dt.np(BF16)
    W1g = np.asarray(W1, f) * np.asarray(g, f).reshape(C, 1)
    # [NMC, 128, CK*256]: W1g[128k+p, 256mc+j] -> [mc, p, (k j)]
    w1h = np.ascontiguousarray(
        W1g.reshape(CK, 128, NMC, 256).transpose(2, 1, 0, 3)
    ).reshape(NMC, 128, CK * 256).astype(bf)
    # [16, 128, 2*1024]: W2[128(2kk+k2)+p, c] -> [kk, p, (k2 c)]
    w2h = np.ascontiguousarray(
        np.asarray(W2, f).reshape(HK // 2, 2, 128, 1024).transpose(0, 2, 1, 3)
    ).reshape(HK // 2, 128, 2 * 1024).astype(bf)
    # [128, HK]: b1t[p, hh] = b1[128hh+p]
    b1t = np.ascontiguousarray(np.asarray(b1, f).reshape(HK, 128).T)
    # [128, CK]: b2t[p, m] = b2[128m+p]
    b2t = np.ascontiguousarray(np.asarray(b2, f).reshape(CK, 128).T)
    return w1h, w2h, b1t, b2t


def _route(u2, centroids):
    """Host-side sigmoid top-2 routing, matching the reference math."""
    f = np.float32
    scores = u2 @ np.asarray(centroids, f)                  # [N, E_R]
    scores = (1.0 / (1.0 + np.exp(-scores))).astype(f)
    order = np.argsort(-scores, axis=1, kind="stable")
    top2 = order[:, :2]                                     # [N, 2]
    den = scores.sum(axis=1)
    gk = (np.take_along_axis(scores, top2, axis=1) / den[:, None]).astype(f)
    return top2, gk


def _prep_inputs(u, g_shared, W1_s, b1_s, W2_s, b2_s,
                 g_routed, W1_r, b1_r, W2_r, b2_r, centroids):
    f = np.float32
    bf = mybir.dt.np(BF16)
    u2 = np.ascontiguousarray(np.asarray(u, f).reshape(B * T, C))
    ss = np.mean(u2.astype(f) * u2, axis=1, dtype=f)
    invr = (1.0 / np.sqrt(ss + EPS)).astype(f)
    xn = u2 * invr[:, None]

    top2, gk = _route(u2, centroids)
    idx_e = [np.nonzero((top2 == e).any(axis=1))[0] for e in range(E_R)]
    gate_e = []
    for e in range(E_R):
        sel = top2[idx_e[e]] == e
        gate_e.append(np.where(sel[:, 0], gk[idx_e[e], 0], gk[idx_e[e], 1]))
    counts = np.array([len(ix) for ix in idx_e])
    cap = int(-(-max(1, counts.max()) // 32) * 32)
    tt = TOKC + cap

    packs = {}
    for g in range(E_S):
        packs["s", g] = _pack_expert(W1_s[g], b1_s[g], W2_s[g], b2_s[g], g_shared)
    for e in range(E_R):
        packs["r", e] = _pack_expert(W1_r[e], b1_r[e], W2_r[e], b2_r[e], g_routed)

    in_maps = []
    for c in range(NCORES):
        g, q = c // 4, c % 4
        w1s, w2s, b1ts, b2ts = packs["s", g]
        w1r, w2r, b1tr, b2tr = packs["r", c]
        ix = idx_e[c]
        npad = cap - len(ix)
        sel = np.concatenate([
            np.arange(TOKC * q, TOKC * (q + 1)),
            ix, np.zeros(npad, np.int64)])
        X = xn[sel]                                          # [tt, C]
        xcore = np.ascontiguousarray(
            X.T.reshape(CK, 128, tt).transpose(1, 0, 2)).reshape(128, CK * tt)
        grow = np.concatenate([
            np.ones(TOKC, f), gate_e[c].astype(f), np.zeros(npad, f)])
        gwc = np.ascontiguousarray(np.broadcast_to(grow[None, :], (128, tt)))
        in_maps.append({
            "xn": xcore.astype(bf),
            "w1": np.stack([w1s, w1r]),
            "w2": np.stack([w2s, w2r]),
            "b1t": np.concatenate([b1ts, b1tr], axis=1),
            "b2c": np.concatenate([b2ts, b2tr], axis=1),
            "gw": gwc,
        })
    aux = dict(cap=cap, idx_e=idx_e, counts=counts, u2=u2)
    return in_maps, aux


def _run(nc, in_maps, trace=False):
    res = bass_utils.run_bass_kernel_spmd(
        nc, in_maps, core_ids=list(range(NCORES)), trace=trace)
    return res


def kernel(**inputs):
    in_maps, aux = _prep_inputs(**inputs)
    cap = aux["cap"]
    tt = TOKC + cap
    key = ("nc", cap)
    if key not in _CACHE:
        _CACHE[key] = _build_program(cap)
    nc = _CACHE[key]
    trace = bool(int(os.environ.get("MOE_TRACE", "0")))
    res = _run(nc, in_maps, trace=trace)
    _CACHE["last_results"] = res

    out2 = aux["u2"].copy()
    for c in range(NCORES):
        q = c % 4
        o = res.results[c]["outT"].reshape(128, CK, tt)
        o = np.ascontiguousarray(o.transpose(1, 0, 2)).reshape(C, tt)
        out2[TOKC * q:TOKC * (q + 1)] += o[:, :TOKC].T
        ix = aux["idx_e"][c]
        out2[ix] += o[:, TOKC:TOKC + len(ix)].T
    return out2.reshape(B, T, C)


# revision 6
# speedup vs baseline: 1.5475x; 1.5475x over previous
"""DeepSeekMoE layer on 8 Trainium2 NeuronCores — host-routed version.

Problem (hardcoded): B=2, T=1024, C=1024, H=4096, E_routed=8 (top-2 sigmoid
gating), E_shared=2, fp32.

The host computes rmsnorm + sigmoid top-2 routing (0.2% of the FLOPs) and
dispatches only the SELECTED (token, expert) work to the device:

  Core c runs two expert segments, SPMD-identical program shape:
    seg 0: shared expert c//4 on token quarter c%4          (512 tokens, bf16)
    seg 1: routed expert c on its selected tokens           (CAP slots, fp8)
  CAP = max routed-expert token count rounded up to 32; unused slots carry
  gate 0.  Total device work drops from 8*5*512 = 20480 token-passes to
  8*(512+CAP) ~ 8700 — a ~2.3x compute reduction vs the dense kernel.

Numerics: the shared experts dominate the output residual, so they stay in
bf16.  The routed experts (gated by ~0.17 weights) run in fp8-e4m3 DoubleRow
(2 contraction chunks per PE pass): weights are scaled by 64 into e4m3's
sweet spot; the scale is undone in the gelu's scale arg (W1 stage) and in
the gate multiplier at eviction (W2 stage).  Measured rel-L2 ~5.6e-3 in a
bit-accurate numpy simulation (tolerance 2e-2).

Device kernel (per core): T-layout [C-partition, token-free].  Host sends
pre-normalized activations xn (g folded into W1 on host, so shared and
routed use the same xn).  Per segment: W1 stationary tiles x xn moving ->
PSUM -> gelu(+b1) on ACT -> h -> W2 stationary x h moving -> PSUM ->
eviction on DVE: out = (psum + b2) * gate (gate pre-broadcast by the host;
1 for shared slots, gate/64 for routed, 0 for pads).  Host scatters the
routed partials back by token index and adds u.
"""
import contextlib
import os
import numpy as np

import concourse.bass as bass
import concourse.tile as tile
from concourse import bacc, mybir
from concourse import bass_utils
from concourse.alu_op_type import AluOpType

F32 = mybir.dt.float32
F32R = mybir.dt.float32r
BF16 = mybir.dt.bfloat16
F8 = mybir.dt.float8e4
AF = mybir.ActivationFunctionType
AX = mybir.AxisListType
DR = mybir.MatmulPerfMode.DoubleRow

B, T, C, H = 2, 1024, 1024, 4096
E_R, E_S = 8, 2
NCORES = 8
TOKC = 512          # shared-segment tokens per core
CK = C // 128       # 8 c-tiles
HK = H // 128       # 32 h-tiles
NMC = 16            # W1 m-chunks (each 2 h-tiles = 256 h cols)
EPS = 1.1920929e-07
WSCL = 64.0         # fp8 weight scale

_CACHE = {}


def _chunks(cap):
    """Token chunks (col offset within the routed segment, size<=512)."""
    out, t0 = [], 0
    while t0 < cap:
        out.append((t0, min(512, cap - t0)))
        t0 += 512
    return out


def _build_program(cap, loop_m=1):
    nc = bacc.Bacc("TRN2", target_bir_lowering=False, debug=False,
                   enable_asserts=False)

    d = {}
    d["xnb"] = nc.dram_tensor("xnb", [128, CK * TOKC], BF16, kind="ExternalInput").ap()
    d["xnr"] = nc.dram_tensor("xnr", [128, CK, cap], F8, kind="ExternalInput").ap()
    d["w1s"] = nc.dram_tensor("w1s", [NMC, 128, CK * 256], BF16, kind="ExternalInput").ap()
    d["w2s"] = nc.dram_tensor("w2s", [HK // 2, 128, 2 * 1024], BF16, kind="ExternalInput").ap()
    d["w1r"] = nc.dram_tensor("w1r", [NMC, 128, CK, 256], F8, kind="ExternalInput").ap()
    d["w2r"] = nc.dram_tensor("w2r", [HK // 2, 128, 2, 1024], F8, kind="ExternalInput").ap()
    d["b1"] = nc.dram_tensor("b1t", [128, 2 * HK], F32, kind="ExternalInput").ap()
    d["b2c"] = nc.dram_tensor("b2c", [128, 2 * CK], F32, kind="ExternalInput").ap()
    d["gw"] = nc.dram_tensor("gw", [128, TOKC + cap], F32, kind="ExternalInput").ap()
    d["out"] = nc.dram_tensor("outT", [128, CK * (TOKC + cap)], F32, kind="ExternalOutput").ap()

    with tile.TileContext(nc) as tc:
        with (
            tc.tile_pool(name="io", bufs=1) as io,
            tc.tile_pool(name="w1p", bufs=2) as w1p,
            tc.tile_pool(name="w2p", bufs=3) as w2p,
            tc.tile_pool(name="hp", bufs=1) as hp,
            tc.tile_pool(name="pp", bufs=8, space="PSUM") as pp,
        ):
            pools = dict(io=io, w1p=w1p, w2p=w2p, hp=hp, pp=pp)
            loop = tc.For_i(0, loop_m, 1) if loop_m > 1 else contextlib.nullcontext()
            with loop:
                _moe_body(nc, d, pools, cap)

    nc.compile()
    return nc


def _moe_body(nc, d, p, cap):
    io, w1p, w2p, hp, pp = p["io"], p["w1p"], p["w2p"], p["hp"], p["pp"]
    tt = TOKC + cap

    # ---- input loads ----
    xnb = io.tile([128, CK * TOKC], BF16, tag="xnb", name="xnb")
    for k in range(CK):
        nc.sync.dma_start(xnb[:, TOKC * k:TOKC * (k + 1)],
                          d["xnb"][:, TOKC * k:TOKC * (k + 1)])
    xnr = io.tile([128, CK, cap], F8, tag="xnr", name="xnr")
    for k in range(0, CK, 2):
        nc.sync.dma_start(xnr[:, k:k + 2, :], d["xnr"][:, k:k + 2, :])
    gw = io.tile([128, tt], F32, tag="gw", name="gw")
    nc.sync.dma_start(gw[:], d["gw"])
    b1 = io.tile([128, 2 * HK], F32, tag="b1", name="b1")
    nc.sync.dma_start(b1[:], d["b1"])
    b2c = io.tile([128, 2 * CK], F32, tag="b2c", name="b2c")
    nc.sync.dma_start(b2c[:], d["b2c"])

    out = io.tile([128, CK * tt], F32, tag="out", name="out")

    def evict(e, py, m, t0, tc_):
        nc.vector.scalar_tensor_tensor(
            out[:, tt * m + t0:tt * m + t0 + tc_], py[:],
            b2c[:, e * CK + m:e * CK + m + 1],
            gw[:, t0:t0 + tc_],
            AluOpType.add, AluOpType.mult)

    # ---- segment 0: shared expert, bf16 ----
    h_act = {}
    for mc in range(NMC):
        w1c = w1p.tile([128, CK * 256], BF16, tag="w1cs", name=f"w1cs_{mc}")
        half = CK * 256 // 2
        nc.sync.dma_start(w1c[:, :half], d["w1s"][mc][:, :half])
        nc.sync.dma_start(w1c[:, half:], d["w1s"][mc][:, half:])
        ph = [pp.tile([128, TOKC], F32, tag="pp", name=f"phs_{mc}_{m}")
              for m in range(2)]
        for k in range(CK):
            for m in range(2):
                nc.tensor.matmul(
                    ph[m][:],
                    w1c[:, 256 * k + 128 * m:256 * k + 128 * (m + 1)],
                    xnb[:, TOKC * k:TOKC * (k + 1)],
                    start=(k == 0), stop=(k == CK - 1))
        for m in range(2):
            hh = 2 * mc + m
            ht = hp.tile([128, TOKC], BF16, tag="hs", bufs=32, name=f"hs_{hh}")
            nc.scalar.activation(ht[:], ph[m][:], AF.Gelu,
                                 bias=b1[:, hh:hh + 1])
            h_act[hh] = ht
    py = [pp.tile([128, TOKC], F32, tag="pp", name=f"pys_{m}")
          for m in range(CK)]
    for kk in range(HK // 2):
        w2c = w2p.tile([128, 2 * 1024], BF16, tag="w2cs", name=f"w2cs_{kk}")
        nc.sync.dma_start(w2c[:, :1024], d["w2s"][kk][:, :1024])
        nc.sync.dma_start(w2c[:, 1024:], d["w2s"][kk][:, 1024:])
        for k2 in range(2):
            for m in range(CK):
                nc.tensor.matmul(
                    py[m][:],
                    w2c[:, 1024 * k2 + 128 * m:1024 * k2 + 128 * (m + 1)],
                    h_act[2 * kk + k2][:],
                    start=(kk == 0 and k2 == 0),
                    stop=(kk == HK // 2 - 1 and k2 == 1))
    for m in range(CK):
        evict(0, py[m], m, 0, TOKC)

    # ---- segment 1: routed expert, fp8 DoubleRow ----
    chunks = _chunks(cap)
    # h for the routed segment lives in one [128, HK, tc] tile per chunk so
    # DoubleRow W2 can address (2kk, 2kk+1) plane pairs with a uniform stride.
    hbig = [hp.tile([128, HK, tc_], F8, tag=f"hr{ci}", name=f"hr_{ci}")
            for ci, (_, tc_) in enumerate(chunks)]
    for mc in range(NMC):
        w1c = w1p.tile([128, CK, 256], F8, tag="w1cr", name=f"w1cr_{mc}")
        nc.sync.dma_start(w1c[:, :CK // 2, :], d["w1r"][mc][:, :CK // 2, :])
        nc.sync.dma_start(w1c[:, CK // 2:, :], d["w1r"][mc][:, CK // 2:, :])
        ph = {(m, ci): pp.tile([128, tc_], F32, tag="pp",
                               name=f"phr_{mc}_{m}_{ci}")
              for m in range(2) for ci, (t0, tc_) in enumerate(chunks)}
        for kp in range(CK // 2):
            for m in range(2):
                for ci, (t0, tc_) in enumerate(chunks):
                    nc.tensor.matmul(
                        ph[m, ci][:],
                        w1c[:, 2 * kp:2 * kp + 2, 128 * m:128 * (m + 1)],
                        xnr[:, 2 * kp:2 * kp + 2, t0:t0 + tc_],
                        start=(kp == 0), stop=(kp == CK // 2 - 1),
                        perf_mode=DR)
        for m in range(2):
            hh = 2 * mc + m
            for ci, (t0, tc_) in enumerate(chunks):
                nc.scalar.activation(
                    hbig[ci][:, hh:hh + 1, :], ph[m, ci][:], AF.Gelu,
                    bias=b1[:, HK + hh:HK + hh + 1], scale=1.0 / WSCL)
    for ci, (t0, tc_) in enumerate(chunks):
        py = [pp.tile([128, tc_], F32, tag="pp", name=f"pyr_{ci}_{m}")
              for m in range(CK)]
        for kk in range(HK // 2):
            w2c = w2p.tile([128, 2, 1024], F8, tag="w2cr", name=f"w2cr_{ci}_{kk}")
            nc.sync.dma_start(w2c[:, :1, :], d["w2r"][kk][:, :1, :])
            nc.sync.dma_start(w2c[:, 1:, :], d["w2r"][kk][:, 1:, :])
            for m in range(CK):
                nc.tensor.matmul(
                    py[m][:],
                    w2c[:, :, 128 * m:128 * (m + 1)],
                    hbig[ci][:, 2 * kk:2 * kk + 2, :],
                    start=(kk == 0), stop=(kk == HK // 2 - 1),
                    perf_mode=DR)
        for m in range(CK):
            evict(1, py[m], m, TOKC + t0, tc_)

    # ---- store (single DMA) ----
    nc.sync.dma_start(d["out"], out[:])


def _pack_expert(W1, b1, W2, b2, g, npdt, scl):
    """Pack one expert's weights for the device layout."""
    f = np.float32
    W1g = np.asarray(W1, f) * np.asarray(g, f).reshape(C, 1) * scl
    # [NMC, 128, CK*256]: W1g[128k+p, 256mc+j] -> [mc, p, (k j)]
    w1h = np.ascontiguousarray(
        W1g.reshape(CK, 128, NMC, 256).transpose(2, 1, 0, 3)
    ).reshape(NMC, 128, CK * 256).astype(npdt)
    # [16, 128, 2*1024]: W2[128(2kk+k2)+p, c] -> [kk, p, (k2 c)]
    w2h = np.ascontiguousarray(
        (np.asarray(W2, f) * scl).reshape(HK // 2, 2, 128, 1024).transpose(0, 2, 1, 3)
    ).reshape(HK // 2, 128, 2 * 1024).astype(npdt)
    # [128, HK]: b1t[p, hh] = b1[128hh+p]
    b1t = np.ascontiguousarray(np.asarray(b1, f).reshape(HK, 128).T)
    # [128, CK]: b2t[p, m] = b2[128m+p] * scl
    b2t = np.ascontiguousarray(np.asarray(b2, f).reshape(CK, 128).T * scl)
    return w1h, w2h, b1t, b2t


def _route(u2, centroids):
    """Host-side sigmoid top-2 routing, matching the reference math."""
    f = np.float32
    scores = u2 @ np.asarray(centroids, f)                  # [N, E_R]
    scores = (1.0 / (1.0 + np.exp(-scores))).astype(f)
    order = np.argsort(-scores, axis=1, kind="stable")
    top2 = order[:, :2]                                     # [N, 2]
    den = scores.sum(axis=1)
    gk = (np.take_along_axis(scores, top2, axis=1) / den[:, None]).astype(f)
    return top2, gk


def _prep_inputs(u, g_shared, W1_s, b1_s, W2_s, b2_s,
                 g_routed, W1_r, b1_r, W2_r, b2_r, centroids):
    f = np.float32
    bf = mybir.dt.np(BF16)
    f8 = mybir.dt.np(F8)
    u2 = np.ascontiguousarray(np.asarray(u, f).reshape(B * T, C))
    ss = np.mean(u2.astype(f) * u2, axis=1, dtype=f)
    invr = (1.0 / np.sqrt(ss + EPS)).astype(f)
    xn = u2 * invr[:, None]

    top2, gk = _route(u2, centroids)
    idx_e = [np.nonzero((top2 == e).any(axis=1))[0] for e in range(E_R)]
    gate_e = []
    for e in range(E_R):
        sel = top2[idx_e[e]] == e
        gate_e.append(np.where(sel[:, 0], gk[idx_e[e], 0], gk[idx_e[e], 1]))
    counts = np.array([len(ix) for ix in idx_e])
    cap = int(-(-max(1, counts.max()) // 32) * 32)
    tt = TOKC + cap

    packs = {}
    for g in range(E_S):
        packs["s", g] = _pack_expert(W1_s[g], b1_s[g], W2_s[g], b2_s[g],
                                     g_shared, bf, 1.0)
    for e in range(E_R):
        packs["r", e] = _pack_expert(W1_r[e], b1_r[e], W2_r[e], b2_r[e],
                                     g_routed, f8, WSCL)

    in_maps = []
    for c in range(NCORES):
        g, q = c // 4, c % 4
        w1s, w2s, b1ts, b2ts = packs["s", g]
        w1r, w2r, b1tr, b2tr = packs["r", c]
        ix = idx_e[c]
        npad = cap - len(ix)
        Xb = xn[TOKC * q:TOKC * (q + 1)]                     # [512, C]
        xnb = np.ascontiguousarray(
            Xb.T.reshape(CK, 128, TOKC).transpose(1, 0, 2)
        ).reshape(128, CK * TOKC).astype(bf)
        selr = np.concatenate([ix, np.zeros(npad, np.int64)])
        Xr = xn[selr]                                        # [cap, C]
        xnr = np.ascontiguousarray(
            Xr.T.reshape(CK, 128, cap).transpose(1, 0, 2)).astype(f8)
        grow = np.concatenate([
            np.ones(TOKC, f), gate_e[c].astype(f) / WSCL, np.zeros(npad, f)])
        gwc = np.ascontiguousarray(np.broadcast_to(grow[None, :], (128, tt)))
        in_maps.append({
            "xnb": xnb,
            "xnr": xnr.reshape(128, CK, cap),
            "w1s": w1s, "w2s": w2s,
            "w1r": w1r.reshape(NMC, 128, CK, 256),
            "w2r": w2r.reshape(HK // 2, 128, 2, 1024),
            "b1t": np.concatenate([b1ts, b1tr], axis=1),
            "b2c": np.concatenate([b2ts, b2tr], axis=1),
            "gw": gwc,
        })
    aux = dict(cap=cap, idx_e=idx_e, counts=counts, u2=u2)
    return in_maps, aux


def _run(nc, in_maps, trace=False):
    res = bass_utils.run_bass_kernel_spmd(
        nc, in_maps, core_ids=list(range(NCORES)), trace=trace)
    return res


def kernel(**inputs):
    in_maps, aux = _prep_inputs(**inputs)
    cap = aux["cap"]
    tt = TOKC + cap
    key = ("nc", cap)
    if key not in _CACHE:
        _CACHE[key] = _build_program(cap)
    nc = _CACHE[key]
    trace = bool(int(os.environ.get("MOE_TRACE", "0")))
    res = _run(nc, in_maps, trace=trace)
    _CACHE["last_results"] = res

    out2 = aux["u2"].copy()
    for c in range(NCORES):
        q = c % 4
        o = res.results[c]["outT"].reshape(128, CK, tt)
        o = np.ascontiguousarray(o.transpose(1, 0, 2)).reshape(C, tt)
        out2[TOKC * q:TOKC * (q + 1)] += o[:, :TOKC].T
        ix = aux["idx_e"][c]
        out2[ix] += o[:, TOKC:TOKC + len(ix)].T
    return out2.reshape(B, T, C)
